# revision 2
# baseline (speedup 1.0000x reference)
"""GAT (3 layers) + global-max-pool + MLP head on 8 Trainium2 NeuronCores.

V2 design: dense-packed gathers + matmul-scatter segment softmax.

Sharding as v1: 64 graphs -> 8 cores, graph slot j padded to common GL[j],
nodes degree-desc sorted within graph, single SPMD NEFF.

Per layer: AllGather h_ext table (bf16 [h(64) | s_src | 0...] 256B rows);
for each 128-node dst tile: gather the tile's incident edges' source rows
DENSELY (per int16 chunk window, count = max over cores, dummy idx 0 pads)
into a [128, J_t, 128] slot grid -- slots are edges in arbitrary order, NOT
node-aligned.  Segment structure is carried by per-core 0/1 matrices:
  M2_j [node, slot]: broadcasts s_dst to slots (PE matmul per 128-col)
  M_j  [slot, node]: scatter-adds P = [num*h | num] into PSUM [128, 65]
num = exp(leaky_relu(s_src + s_dst)) with NO per-segment max subtraction
(e in [-0.9, 4.6] for this data; softmax is shift-invariant so this is
mathematically identical to the reference).  h = out[:, :64]/(out[:,64]+eps).

This cuts gathered rows/layer/core from ~612k (v1 rank-grid padding 4.4x)
to ~141k (dense, ~3% cross-core padding), removing the dominant cost.
"""

import os
import sys
import numpy as np

for _p in ("/opt/trn_rl_repo", "/opt/trn_rl_repo/concourse"):
    if _p not in sys.path:
        sys.path.insert(0, _p)

import concourse.bass as bass  # noqa: E402
import concourse.bacc as bacc  # noqa: E402
import concourse.mybir as mybir  # noqa: E402
import concourse.tile as tile  # noqa: E402
from concourse.bass_utils import run_bass_kernel_spmd  # noqa: E402

F32 = mybir.dt.float32
BF16 = mybir.dt.bfloat16
F8 = mybir.dt.float8e4
I16 = mybir.dt.int16
ALU = mybir.AluOpType
ACTF = mybir.ActivationFunctionType
AX = mybir.AxisListType

NCORES = 8
NGRAPH = 64
CHUNK = 32768
ROW = 128
NQ = int(os.environ.get("GAT_NQ", "1"))       # SWDGE queues for gathers


def _ap(t, off, dims):
    return bass.AP(t, off, dims)


# ----------------------------------------------------------------------------
# Host-side preprocessing
# ----------------------------------------------------------------------------

def _preprocess(adj, batch):
    N = batch.shape[0]
    gper = NGRAPH // NCORES
    graph_of = batch.astype(np.int64)
    counts = np.bincount(graph_of, minlength=NGRAPH)
    gstarts = np.zeros(NGRAPH + 1, np.int64)
    np.cumsum(counts, out=gstarts[1:])

    src = np.concatenate([adj[0].astype(np.int64),
                          np.arange(N, dtype=np.int64)])
    dst = np.concatenate([adj[1].astype(np.int64),
                          np.arange(N, dtype=np.int64)])
    deg = np.bincount(dst, minlength=N)

    glens = counts.reshape(NCORES, gper)
    GL = np.maximum(glens.max(axis=0), 1)
    GST = np.zeros(gper + 1, np.int64)
    np.cumsum(GL, out=GST[1:])
    NPADC = int(np.ceil(GST[-1] / 128) * 128)
    NT = NPADC // 128

    order = np.lexsort((-deg, graph_of))
    new_of_old = np.empty(N, np.int64)
    order_padded = np.full((NCORES, NPADC), -1, np.int64)
    for g in range(NGRAPH):
        c, j = g // gper, g % gper
        olds = order[gstarts[g]:gstarts[g + 1]]
        col0 = GST[j]
        order_padded[c, col0:col0 + len(olds)] = olds
        new_of_old[olds] = c * NPADC + col0 + np.arange(len(olds))

    NTOT = NCORES * NPADC
    NCH = int((NTOT + CHUNK - 1) // CHUNK)

    nsrc = new_of_old[src]
    ndst = new_of_old[dst]
    dst_core = ndst // NPADC
    dst_local = ndst % NPADC
    ch = nsrc // CHUNK
    lo = (nsrc % CHUNK).astype(np.int64)
    til = dst_local // 128
    p_dst = dst_local % 128

    # shared slot counts per (tile, chunk): max over cores
    cnt = np.zeros((NCORES, NT, NCH), np.int64)
    np.add.at(cnt, (dst_core, til, ch), 1)
    C = cnt.max(axis=0)                      # [NT, NCH]
    ncol = (C + 127) // 128                  # gather columns per (t, ch)
    J = ncol.sum(axis=1)                     # [NT]
    JMAX = int(J.max())
    TB = np.zeros(NT + 1, np.int64)
    np.cumsum(J, out=TB[1:])
    TJ = int(TB[-1])
    O = np.zeros((NT, NCH), np.int64)        # col offset of chunk in tile
    O[:, 1:] = np.cumsum(ncol[:, :-1], axis=1)

    # idx table: per-tile contiguous [chunk0 cols | chunk1 cols | ...]
    icn = (C + 15) // 16                     # idx cols per (t, ch)
    itile = icn.sum(axis=1)                  # idx cols per tile
    IBt = np.zeros(NT + 1, np.int64)
    np.cumsum(itile, out=IBt[1:])
    ICOLS = int(IBt[-1])
    IB = np.zeros((NT, NCH), np.int64)
    IB[:, 0] = IBt[:-1]
    IB[:, 1:] = IBt[:-1, None] + np.cumsum(icn[:, :-1], axis=1)

    f8np = mybir.dt.np(F8)
    idx_tabs, m_tabs, m2_tabs, vlds = [], [], [], []
    for c in range(NCORES):
        m = dst_core == c
        o = np.lexsort((lo[m], dst_local[m], ch[m], til[m]))
        tl = til[m][o]
        cc = ch[m][o]
        pp_ = p_dst[m][o]
        ll = lo[m][o]
        ne = len(tl)
        # rank within (tile, chunk) group
        keys = tl * NCH + cc
        brk = np.ones(ne, bool)
        brk[1:] = keys[1:] != keys[:-1]
        gid = np.cumsum(brk) - 1
        gst = np.zeros(gid[-1] + 2, np.int64)
        np.add.at(gst[1:], gid, 1)
        np.cumsum(gst, out=gst)
        q = np.arange(ne) - gst[gid]         # position within the call

        it = np.zeros((16, ICOLS), np.int16)
        it[q % 16, IB[tl, cc] + q // 16] = ll.astype(np.int16)
        idx_tabs.append(np.tile(it, (8, 1)))

        jcol = TB[tl] + O[tl, cc] + q // 128  # global slot column
        p_slot = q % 128
        mt = np.zeros((128, TJ * 128), f8np)
        mt[p_slot, jcol * 128 + pp_] = f8np(1.0)
        m_tabs.append(mt)
        m2 = np.zeros((128, TJ * 128), f8np)
        m2[pp_, jcol * 128 + p_slot] = f8np(1.0)
        m2_tabs.append(m2)

        vld = np.zeros((128, NT), np.float32)
        padm = order_padded[c] < 0
        for ti in range(NT):
            vld[:, ti] = (~padm[ti * 128:(ti + 1) * 128]).astype(np.float32)
        vlds.append(vld)

    return dict(
        N=N, gper=gper, NPADC=NPADC, NT=NT, NTOT=NTOT, NCH=NCH,
        order_padded=order_padded,
        C=C, ncol=ncol, J=J, JMAX=JMAX, TB=TB, TJ=TJ, O=O,
        icn=icn, IB=IB, IBt=IBt, ICOLS=ICOLS,
        idx_tabs=idx_tabs, m_tabs=m_tabs, m2_tabs=m2_tabs, vlds=vlds,
        GL=[int(v) for v in GL], GST=[int(v) for v in GST],
        roots=gstarts[:NGRAPH].copy(),
    )


# ----------------------------------------------------------------------------
# Device program
# ----------------------------------------------------------------------------

def _build_program(pp, IN, HID):
    NPADC, NT, NTOT, NCH = pp["NPADC"], pp["NT"], pp["NTOT"], pp["NCH"]
    C, ncol, J, JMAX, TB, TJ, O = (pp["C"], pp["ncol"], pp["J"], pp["JMAX"],
                                   pp["TB"], pp["TJ"], pp["O"])
    icn, IB, IBt, ICOLS = pp["icn"], pp["IB"], pp["IBt"], pp["ICOLS"]
    GL, GST, gper = pp["GL"], pp["GST"], pp["gper"]
    GLMAX = int(np.ceil(max(GL) / 128) * 128)
    HE = HID + 1                              # h features + num column

    nc = bacc.Bacc("TRN2", target_bir_lowering=False, debug=False,
                   num_devices=NCORES, num_swdge_queues=NQ)

    xT = nc.dram_tensor("xT", [IN, NPADC], F32, kind="ExternalInput")
    xrootT = nc.dram_tensor("xrootT", [IN, gper], F32, kind="ExternalInput")
    idx_t = nc.dram_tensor("idx", [128, ICOLS], I16, kind="ExternalInput")
    m_t = nc.dram_tensor("mtab", [128, TJ * 128], F8, kind="ExternalInput")
    m2_t = nc.dram_tensor("m2tab", [128, TJ * 128], F8, kind="ExternalInput")
    vld_t = nc.dram_tensor("vld", [128, NT], F32, kind="ExternalInput")
    Ws = {}
    for l, di in ((1, IN), (2, HID), (3, HID)):
        Ws[f"W{l}"] = nc.dram_tensor(f"W{l}", [di, HID], F32,
                                     kind="ExternalInput")
        Ws[f"as{l}"] = nc.dram_tensor(f"as{l}", [HID, 1], F32,
                                      kind="ExternalInput")
        Ws[f"ad{l}"] = nc.dram_tensor(f"ad{l}", [HID, 1], F32,
                                      kind="ExternalInput")
        Ws[f"b{l}"] = nc.dram_tensor(f"b{l}", [128, HID], F32,
                                     kind="ExternalInput")
    lin0W = nc.dram_tensor("lin0W", [HID, HID], F32, kind="ExternalInput")
    lin0b = nc.dram_tensor("lin0b", [gper, HID], F32, kind="ExternalInput")
    linnW = nc.dram_tensor("linnW", [IN, HID], F32, kind="ExternalInput")
    linnb = nc.dram_tensor("linnb", [gper, HID], F32, kind="ExternalInput")
    lin1W = nc.dram_tensor("lin1W", [2 * HID, 1], F32, kind="ExternalInput")
    lin1b = nc.dram_tensor("lin1b", [gper, 1], F32, kind="ExternalInput")
    ident_in = nc.dram_tensor("ident", [128, 128], F32, kind="ExternalInput")
    out_t = nc.dram_tensor("out", [gper, 1], F32, kind="ExternalOutput")

    agin = [nc.dram_tensor(f"agin{l}", [NPADC, ROW], BF16, kind="Internal")
            for l in range(3)]
    htab = [nc.dram_tensor(f"htab{l}", [NTOT, ROW], BF16, kind="Internal",
                           addr_space="Shared")
            for l in range(3)]
    x4T_d = nc.dram_tensor("x4T", [HID, NPADC], F32, kind="Internal")

    with tile.TileContext(nc) as tc:
        with (
            tc.tile_pool(name="const", bufs=1) as cpool,
            tc.tile_pool(name="gbuf", bufs=2) as gpool,
            tc.tile_pool(name="mbuf", bufs=2) as mpool,
            tc.tile_pool(name="pbuf", bufs=2) as ppool,
            tc.tile_pool(name="sbuf", bufs=3) as spool,
            tc.tile_pool(name="psum", bufs=2, space="PSUM") as pspool,
            tc.tile_pool(name="psA", bufs=2, space="PSUM") as psA,
        ):
            ident = cpool.tile([128, 128], F32, tag="ident")
            nc.sync.dma_start(ident[:], ident_in[:])
            # init G buffers to 0 (stale partial-column slots must be finite)
            for _k in range(2):
                Gz = gpool.tile([128, JMAX, ROW], BF16, tag="G")
                nc.vector.memset(Gz[:], 0.0)

            # Wcat_l = [W_l | W_l@a_src | W_l@a_dst], bias broadcasts
            wcat = []
            s_dst_res = []
            for l, di in ((1, IN), (2, HID), (3, HID)):
                w_sb = cpool.tile([di, HID], F32, tag=f"w{l}")
                nc.sync.dma_start(w_sb[:], Ws[f"W{l}"][:])
                wc = cpool.tile([di, HID + 2], F32, tag=f"wc{l}")
                nc.vector.tensor_copy(wc[:, :HID], w_sb[:])
                ps_wt = psA.tile([HID, 128], F32, space="PSUM", tag="aux",
                                 name="ps_wt")
                nc.tensor.transpose(ps_wt[:, :di], w_sb[:], ident[:di, :di])
                wt_sb = cpool.tile([HID, 128], F32, tag=f"wt{l}")
                nc.scalar.copy(wt_sb[:, :di], ps_wt[:, :di])
                for name, col in ((f"as{l}", HID), (f"ad{l}", HID + 1)):
                    a_sb = cpool.tile([HID, 1], F32, tag=f"t{name}")
                    nc.sync.dma_start(a_sb[:], Ws[name][:])
                    ps_wa = psA.tile([128, 1], F32, space="PSUM", tag="aux",
                                     name="ps_wa")
                    nc.tensor.matmul(ps_wa[:di, :], wt_sb[:, :di], a_sb[:])
                    nc.vector.tensor_copy(wc[:, col:col + 1], ps_wa[:di, :])
                wcat.append(wc)
                b_sb = cpool.tile([128, HID], F32, tag=f"bb{l}")
                nc.sync.dma_start(b_sb[:], Ws[f"b{l}"][:])
                Ws[f"bsb{l}"] = b_sb
                s_dst_res.append(cpool.tile([128, NT], F32, tag=f"sdst{l}",
                                            name=f"sdst{l}"))

            vld_sb = cpool.tile([128, NT], F32, tag="vld")
            nc.sync.dma_start(vld_sb[:], vld_t[:])

            # phase A: layer-1 h_ext rows from x
            for t in range(NT):
                x_sb = spool.tile([IN, 128], F32, tag="ax")
                nc.sync.dma_start(x_sb[:], xT[:, t * 128:(t + 1) * 128])
                ps_h = psA.tile([128, HID + 2], F32, space="PSUM", tag="ph",
                                name="ps_h")
                nc.tensor.matmul(ps_h[:], x_sb[:], wcat[0][:])
                hx = spool.tile([128, ROW], BF16, tag="hx")
                nc.vector.memset(hx[:, HID + 1:], 0.0)
                nc.scalar.copy(hx[:, :HID + 1], ps_h[:, :HID + 1])
                nc.vector.tensor_copy(s_dst_res[0][:, t:t + 1],
                                      ps_h[:, HID + 1:HID + 2])
                nc.sync.dma_start(agin[0][t * 128:(t + 1) * 128, :], hx[:])

            # 3 GAT layers
            for l in range(3):
                nc.gpsimd.collective_compute(
                    "AllGather", ALU.bypass,
                    replica_groups=[list(range(NCORES))],
                    ins=[agin[l][:]], outs=[htab[l][:]],
                )
                for t in range(NT):
                    Jt = int(J[t])
                    # gather the tile's edges (dense, per chunk window)
                    ix = spool.tile([128, int(icn[t].sum())], I16, tag="ix",
                                    padded_shape=[128, int(icn.sum(1).max())])
                    nc.sync.dma_start(
                        ix[:], idx_t[:, int(IBt[t]):int(IBt[t + 1])])
                    G_sb = gpool.tile([128, JMAX, ROW], BF16, tag="G")
                    goff = G_sb[:].offset
                    ioff = 0
                    for chn in range(NCH):
                        Ct = int(C[t, chn])
                        if Ct == 0:
                            continue
                        rows_c = min(CHUNK, NTOT - chn * CHUNK)
                        in_ap = _ap(htab[l], chn * CHUNK * ROW,
                                    [(ROW, rows_c), (1, ROW)])
                        out_ap = _ap(
                            G_sb.tensor, goff + int(O[t, chn]) * ROW,
                            [(JMAX * ROW, 128), (ROW, int(ncol[t, chn])),
                             (1, ROW)])
                        nc.gpsimd.dma_gather(
                            out_ap, in_ap,
                            ix[:, ioff:ioff + int(icn[t, chn])],
                            Ct, Ct, ROW, single_packet=False,
                            queue_num=(t * NCH + chn) % NQ)
                        ioff += int(icn[t, chn])

                    # M tables for this tile
                    M_sb = mpool.tile([128, Jt * 128], F8, tag="M",
                                      padded_shape=[128, JMAX * 128])
                    nc.sync.dma_start(
                        M_sb[:],
                        m_t[:, int(TB[t]) * 128:int(TB[t + 1]) * 128])
                    M2_sb = mpool.tile([128, Jt * 128], F8, tag="M2",
                                       padded_shape=[128, JMAX * 128])
                    nc.sync.dma_start(
                        M2_sb[:],
                        m2_t[:, int(TB[t]) * 128:int(TB[t + 1]) * 128])

                    # one PSUM tile: cols [0:HE) = scatter accum,
                    # cols [HE:HE+Jt) = s_dst broadcast to slots
                    ps_o = psA.tile([128, HE + JMAX], F32, space="PSUM",
                                    tag="po", name="ps_o")
                    sd_b = spool.tile([128, 1], BF16, tag="sd")
                    nc.vector.tensor_copy(sd_b[:], s_dst_res[l][:, t:t + 1])
                    for j in range(Jt):
                        nc.tensor.matmul(ps_o[:, HE + j:HE + j + 1],
                                         M2_sb[:, j * 128:(j + 1) * 128],
                                         sd_b[:])

                    # num = exp(leaky_relu(s_src + s_dst))
                    e_sb = spool.tile([128, JMAX], F32, tag="e")
                    ssrc_v = _ap(G_sb.tensor, goff + HID,
                                 [(JMAX * ROW, 128), (ROW, Jt)])
                    nc.vector.tensor_tensor(e_sb[:, :Jt], ssrc_v,
                                            ps_o[:, HE:HE + Jt], ALU.add)
                    e2_sb = spool.tile([128, JMAX], F32, tag="e2")
                    nc.scalar.activation(e2_sb[:, :Jt], e_sb[:, :Jt],
                                         ACTF.Copy, scale=0.2)
                    nc.vector.tensor_tensor(e_sb[:, :Jt], e_sb[:, :Jt],
                                            e2_sb[:, :Jt], ALU.max)
                    nb = spool.tile([128, JMAX], BF16, tag="nb")
                    nc.scalar.activation(nb[:, :Jt], e_sb[:, :Jt], ACTF.Exp)

                    # P = [num*h | num]
                    P_sb = ppool.tile([128, JMAX, HE], BF16, tag="P")
                    poff = P_sb[:].offset
                    gv = _ap(G_sb.tensor, goff,
                             [(JMAX * ROW, 128), (ROW, Jt), (1, HID)])
                    nbv = _ap(nb.tensor, nb[:].offset,
                              [(JMAX, 128), (1, Jt), (0, HID)])
                    pv = _ap(P_sb.tensor, poff,
                             [(JMAX * HE, 128), (HE, Jt), (1, HID)])
                    nc.any.tensor_tensor(pv, gv, nbv, ALU.mult)
                    pnv = _ap(P_sb.tensor, poff + HID,
                              [(JMAX * HE, 128), (HE, Jt)])
                    nc.vector.tensor_copy(pnv, nb[:, :Jt])

                    # scatter-accumulate into [node, h|den]
                    for j in range(Jt):
                        nc.tensor.matmul(
                            ps_o[:, :HE],
                            M_sb[:, j * 128:(j + 1) * 128],
                            _ap(P_sb.tensor, poff + j * HE,
                                [(JMAX * HE, 128), (1, HE)]),
                            start=(j == 0), stop=(j == Jt - 1))

                    # h = out/(den+eps) + b, relu
                    den = spool.tile([128, 1], F32, tag="den")
                    nc.scalar.activation(den[:], ps_o[:, HID:HID + 1],
                                         ACTF.Copy, bias=1e-16)
                    rec = spool.tile([128, 1], F32, tag="rec")
                    nc.vector.reciprocal(rec[:], den[:])
                    o_sb = spool.tile([128, HID], F32, tag="o")
                    recb = _ap(rec.tensor, rec[:].offset,
                               [(1, 128), (0, HID)])
                    nc.vector.tensor_tensor(o_sb[:], ps_o[:, :HID], recb,
                                            ALU.mult)
                    bsb = Ws[f"bsb{l + 1}"]
                    nc.vector.tensor_tensor(o_sb[:], o_sb[:], bsb[:],
                                            ALU.add)
                    nc.scalar.activation(o_sb[:], o_sb[:], ACTF.Relu)
                    if l == 2:
                        vb = _ap(vld_sb.tensor, vld_sb[:].offset + t,
                                 [(NT, 128), (0, HID)])
                        nc.vector.tensor_tensor(o_sb[:], o_sb[:], vb,
                                                ALU.mult)

                    # tail: transpose + next-layer fused projection
                    ps_t = pspool.tile([HID, 128], F32, space="PSUM")
                    nc.tensor.transpose(ps_t[:], o_sb[:], ident[:])
                    xt_sb = spool.tile([HID, 128], F32, tag="xt")
                    nc.scalar.copy(xt_sb[:], ps_t[:])
                    if l < 2:
                        ps_h = psA.tile([128, HID + 2], F32, space="PSUM",
                                        tag="ph", name="ps_h")
                        nc.tensor.matmul(ps_h[:], xt_sb[:], wcat[l + 1][:])
                        hx = spool.tile([128, ROW], BF16, tag="hx")
                        nc.vector.memset(hx[:, HID + 1:], 0.0)
                        nc.scalar.copy(hx[:, :HID + 1], ps_h[:, :HID + 1])
                        nc.vector.tensor_copy(
                            s_dst_res[l + 1][:, t:t + 1],
                            ps_h[:, HID + 1:HID + 2])
                        nc.sync.dma_start(
                            agin[l + 1][t * 128:(t + 1) * 128, :], hx[:])
                    else:
                        nc.sync.dma_start(
                            x4T_d[:, t * 128:(t + 1) * 128], xt_sb[:])

            # head (identical to v1)
            hmaxT = cpool.tile([HID, gper], F32, tag="hmaxT")
            for g in range(gper):
                hg = spool.tile([HID, GLMAX], F32, tag="hg")
                nc.sync.dma_start(hg[:, :GL[g]],
                                  x4T_d[:, GST[g]:GST[g] + GL[g]])
                nc.vector.tensor_reduce(hmaxT[:, g:g + 1], hg[:, :GL[g]],
                                        AX.X, ALU.max)
            lw_sb = cpool.tile([HID, HID], F32, tag="l0w")
            nc.sync.dma_start(lw_sb[:], lin0W[:])
            ps_g = psA.tile([gper, HID], F32, space="PSUM", tag="aux",
                            name="ps_g")
            nc.tensor.matmul(ps_g[:], hmaxT[:], lw_sb[:])
            b0_sb = cpool.tile([gper, HID], F32, tag="l0b")
            nc.sync.dma_start(b0_sb[:], lin0b[:])
            h0 = cpool.tile([gper, HID], F32, tag="h0")
            nc.vector.tensor_tensor(h0[:], ps_g[:], b0_sb[:], ALU.add)
            nc.scalar.activation(h0[:], h0[:], ACTF.Relu)

            xr_sb = cpool.tile([IN, gper], F32, tag="xr")
            nc.sync.dma_start(xr_sb[:], xrootT[:])
            lnw_sb = cpool.tile([IN, HID], F32, tag="lnw")
            nc.sync.dma_start(lnw_sb[:], linnW[:])
            ps_n = psA.tile([gper, HID], F32, space="PSUM", tag="aux",
                            name="ps_n")
            nc.tensor.matmul(ps_n[:], xr_sb[:], lnw_sb[:])
            bn_sb = cpool.tile([gper, HID], F32, tag="lnb")
            nc.sync.dma_start(bn_sb[:], linnb[:])
            hn = cpool.tile([gper, HID], F32, tag="hn")
            nc.vector.tensor_tensor(hn[:], ps_n[:], bn_sb[:], ALU.add)
            nc.scalar.activation(hn[:], hn[:], ACTF.Relu)

            catT = cpool.tile([2 * HID, gper], F32, tag="catT")
            ps_t0 = psA.tile([HID, gper], F32, space="PSUM", tag="aux",
                             name="ps_t0")
            nc.tensor.transpose(ps_t0[:], h0[:], ident[:gper, :gper])
            nc.scalar.copy(catT[:HID, :], ps_t0[:])
            ps_t1 = psA.tile([HID, gper], F32, space="PSUM", tag="aux",
                             name="ps_t1")
            nc.tensor.transpose(ps_t1[:], hn[:], ident[:gper, :gper])
            nc.scalar.copy(catT[HID:, :], ps_t1[:])

            l1w_sb = cpool.tile([2 * HID, 1], F32, tag="l1w")
            nc.sync.dma_start(l1w_sb[:], lin1W[:])
            ps_fo = psA.tile([gper, 1], F32, space="PSUM", tag="aux",
                             name="ps_fo")
            nc.tensor.matmul(ps_fo[:], catT[:], l1w_sb[:])
            b1_sb = cpool.tile([gper, 1], F32, tag="l1b")
            nc.sync.dma_start(b1_sb[:], lin1b[:])
            o_fin = cpool.tile([gper, 1], F32, tag="ofin")
            nc.scalar.activation(o_fin[:], ps_fo[:], ACTF.Sigmoid,
                                 bias=b1_sb[:])
            nc.sync.dma_start(out_t[:], o_fin[:])

    nc.compile()
    return nc


# ----------------------------------------------------------------------------
# entry point
# ----------------------------------------------------------------------------

_CACHE = {}
LAST_RESULTS = None
LAST_NC = None
LAST_INMAPS = None


def kernel(x, adj, batch, W1, a_src1, a_dst1, b1, W2, a_src2, a_dst2, b2,
           W3, a_src3, a_dst3, b3, linnews_W, linnews_b, lin0_W, lin0_b,
           lin1_W, lin1_b):
    x = np.asarray(x)
    adj = np.asarray(adj)
    batch = np.asarray(batch)
    N, IN = x.shape
    HID = np.asarray(W1).shape[1]
    gper = NGRAPH // NCORES

    ckey = (N, adj.shape[1], IN, HID,
            hash(adj.tobytes()), hash(batch.tobytes()))
    if ckey in _CACHE:
        pp, nc = _CACHE[ckey]
    else:
        pp = _preprocess(adj, batch)
        nc = _build_program(pp, IN, HID)
        _CACHE.clear()
        _CACHE[ckey] = (pp, nc)

    NPADC = pp["NPADC"]
    order_padded = pp["order_padded"]
    f32 = np.float32
    in_maps = []
    for c in range(NCORES):
        oc = order_padded[c]
        xc = np.zeros((NPADC, IN), f32)
        real = oc >= 0
        xc[real] = np.asarray(x, f32)[oc[real]]
        roots = pp["roots"][c * gper:(c + 1) * gper]
        im = {
            "xT": np.ascontiguousarray(xc.T),
            "xrootT": np.ascontiguousarray(np.asarray(x, f32)[roots].T),
            "idx": pp["idx_tabs"][c],
            "mtab": pp["m_tabs"][c],
            "m2tab": pp["m2_tabs"][c],
            "vld": pp["vlds"][c],
            "W1": np.asarray(W1, f32), "W2": np.asarray(W2, f32),
            "W3": np.asarray(W3, f32),
            "as1": np.asarray(a_src1, f32).reshape(HID, 1),
            "ad1": np.asarray(a_dst1, f32).reshape(HID, 1),
            "as2": np.asarray(a_src2, f32).reshape(HID, 1),
            "ad2": np.asarray(a_dst2, f32).reshape(HID, 1),
            "as3": np.asarray(a_src3, f32).reshape(HID, 1),
            "ad3": np.asarray(a_dst3, f32).reshape(HID, 1),
            "b1": np.tile(np.asarray(b1, f32).reshape(1, HID), (128, 1)),
            "b2": np.tile(np.asarray(b2, f32).reshape(1, HID), (128, 1)),
            "b3": np.tile(np.asarray(b3, f32).reshape(1, HID), (128, 1)),
            "lin0W": np.asarray(lin0_W, f32),
            "lin0b": np.tile(np.asarray(lin0_b, f32).reshape(1, HID),
                             (gper, 1)),
            "linnW": np.asarray(linnews_W, f32),
            "linnb": np.tile(np.asarray(linnews_b, f32).reshape(1, HID),
                             (gper, 1)),
            "lin1W": np.asarray(lin1_W, f32).reshape(2 * HID, 1),
            "lin1b": np.tile(np.asarray(lin1_b, f32).reshape(1, 1),
                             (gper, 1)),
            "ident": np.eye(128, dtype=f32),
        }
        in_maps.append(im)

    global LAST_RESULTS, LAST_NC, LAST_INMAPS
    LAST_NC, LAST_INMAPS = nc, in_maps
    res = run_bass_kernel_spmd(nc, in_maps, core_ids=list(range(NCORES)))
    LAST_RESULTS = res
    out = np.concatenate([res.results[c]["out"] for c in range(NCORES)],
                         axis=0)
    return out.astype(np.float32)
